# revision 15
# baseline (speedup 1.0000x reference)
"""Trainium2 Bass kernel for nn_LstmCellS (matrix-state LSTM cell).

Math (per gate g in [f, i, o, c]):
    pre[g] = hidden_u @ Ww[g]^T - x @ Wd[g]^T + hidden_s @ Wu[g]^T + (Bw+Bd+Bu)[g]
    f, i, o = sigmoid(pre[0..2]);  gg = tanh(pre[3])
    c     = f * hidden_c + i * gg
    out_s = o * tanh(c)

Sharding: tensor-parallel over the output axis p (flattened (a, b), S^2 = 4096
-> 512 per core).  Every core sees the full batch and full contraction but only
its 512-wide output slice of every gate, so the whole LSTM epilogue is local —
no collectives.  Host concatenates the 8 output slices.

Mixed precision (rel err ~1.77e-2 vs the 2e-2 budget; inputs deterministic):
  * f/i/o gates run fp8-e4m3 DoubleRow (2 contraction rows per PE cell/cycle).
  * The candidate (tanh) gate keeps bf16 for the first QB=20 hidden_s k-tiles
    and fp8 DoubleRow for the rest, reusing the resident fp8 activations.
  * fp8 operands pre-scaled on host (acts x16, weights x4096); bf16 operands
    carry the same power-of-two scales so every PSUM bank is uniformly
    2^16-scaled and the epilogue activation applies scale=2^-16.

Schedule (v2, from trace analysis of the v1 kernel):
  * ~7us of framework preamble is fixed; first DMA packets land ~8.4us.  DMA
    sustains ~400 GB/s; the PE matmul stream (~40us warm) is the critical
    path, so the schedule exists to keep the PE fed from the first packet on.
  * DMA issue order == need order.  sync queue carries all weights
    (w8f slabs fine-grained for JIT start, then w8i, wg8, wbg, w8o, then
    result stores); scalar queue carries bias + a8 chunks (JIT for the
    f-pass), then stalls behind the f-sigmoids so ab/hc cannot preempt w8i.
  * The f-pass is DMA-ramp-paced, so the 8 bias matmuls (one K=1 matmul per
    PSUM bank) are slotted between the first f-pairs where the PE has slack;
    banks not yet opened get their bias matmul as the start=True group head.
  * Only 5 dummy matmuls bridge engine-free (~7us) to first-weights (~9us);
    cold-clock matmuls are faster than the early DMA pace anyway.
  * Gate order f, i, candidate, o: c = f*hc + i*g runs on scalar/vector under
    the o matmuls.  o is n-major; batch-tile 1 is column-split so its first
    half's sigmoid/mul/store hides under the second half's matmuls.
"""

import sys

for _p in ("/root/.axon_site/_ro/trn_rl_repo", "/opt/trn_rl_repo"):
    if _p not in sys.path:
        sys.path.append(_p)

import ml_dtypes
import numpy as np

B = 256          # batch
S2 = 4096        # S*S (flattened matrix state)
U = 512          # hidden_u size
I = 512          # input size
QC = S2 + U + I  # contraction length (5120)
QT = QC // 128   # contraction tiles (40)
KP = QT // 2     # fp8 DoubleRow k-pairs (20)
QB = 10          # candidate-gate bf16 k-tiles (high-variance section)
KG = (QT - QB) // 2  # candidate-gate fp8 k-pairs (10)
NT = B // 128    # batch tiles (2)
NCORES = 8
PSH = S2 // NCORES  # output slice per core (512)

SA = 16.0        # fp8 activation scale
SW = 4096.0      # fp8 weight scale
SINV = 1.0 / (SA * SW)

F8 = ml_dtypes.float8_e4m3  # TRN float8e4: bias 7, max normal +-240
BF = ml_dtypes.bfloat16

_cache: dict = {}


def _build():
    """Build and compile the per-core Bass module (same NEFF on all cores)."""
    import concourse.tile as tile
    import concourse.mybir as mybir
    from concourse import bacc

    f32 = mybir.dt.float32
    bf16 = mybir.dt.bfloat16
    fp8 = mybir.dt.float8e4
    AF = mybir.ActivationFunctionType
    DR = mybir.MatmulPerfMode.DoubleRow

    nc = bacc.Bacc("TRN2", target_bir_lowering=False, debug=False,
                   enable_asserts=False, num_devices=NCORES)

    A8_d = nc.dram_tensor("A8", [128, QT * B], fp8, kind="ExternalInput")
    AB_d = nc.dram_tensor("AB", [128, QB * B], bf16, kind="ExternalInput")
    W8_d = nc.dram_tensor("W8", [3, 128, KP * 2 * PSH], fp8, kind="ExternalInput")
    WG_d = nc.dram_tensor("WG8", [128, KG * 2 * PSH], fp8, kind="ExternalInput")
    WB_d = nc.dram_tensor("WB", [128, QB * PSH], bf16, kind="ExternalInput")
    B_d = nc.dram_tensor("BIAS", [1, 4 * PSH], bf16, kind="ExternalInput")
    H_d = nc.dram_tensor("HC", [128, NT * PSH], bf16, kind="ExternalInput")
    OS_d = nc.dram_tensor("OS", [NT, 128, PSH], bf16, kind="ExternalOutput")
    CO_d = nc.dram_tensor("CO", [NT, 128, PSH], bf16, kind="ExternalOutput")

    with tile.TileContext(nc) as tc:
        with (
            tc.tile_pool(name="apool", bufs=1) as apool,
            tc.tile_pool(name="wpool", bufs=1) as wpool,
            tc.tile_pool(name="cpool", bufs=1) as cpool,
            tc.tile_pool(name="epool", bufs=2) as epool,
            tc.tile_pool(name="pspool", bufs=1, space="PSUM") as pspool,
        ):
            # PSUM accumulators: bank per (gate, batch-tile); gate ids f0 i1 o2 g3
            psum = [
                pspool.tile([128, PSH], f32, tag=f"ps{g}_{n}", name=f"ps{g}_{n}")
                for g in range(4) for n in range(NT)
            ]

            # Contiguous dummy-matmul block: ~3.8us of back-to-back PE work
            # flips the HAM clock gate to 8/8 by ~11us, so the whole f-pass
            # runs at 2.4 GHz instead of 1.2 (a sparse early stream with DMA
            # waits never sustains a full HAM busy window — measured warm-up
            # then happens only at ~21us, leaving the f-pass cold-PE-bound).
            # They pollute psum[0], which the f start=True head clears.
            scr = cpool.tile([128, 128 + PSH], bf16, tag="scr")
            nc.vector.memset(scr[:], 0.0)
            for _ in range(9):
                nc.tensor.matmul(
                    psum[0][:], scr[:, :128], scr[:, 128:],
                    start=True, stop=True, skip_group_check=True)

            # SBUF-resident operands.
            a8 = apool.tile([128, QT, B], fp8, tag="a8", name="a8")
            ab = apool.tile([128, QB * B], bf16, tag="ab", name="ab")
            wg8 = wpool.tile([128, KG, 2, PSH], fp8, tag="wg8", name="wg8")
            w8f = wpool.tile([128, KP, 2, PSH], fp8, tag="w8f", name="w8f")
            w8i = wpool.tile([128, KP, 2, PSH], fp8, tag="w8i", name="w8i")
            w8o = wpool.tile([128, KP, 2, PSH], fp8, tag="w8o", name="w8o")
            wbg = wpool.tile([128, QB * PSH], bf16, tag="wbg", name="wbg")
            hc_t = cpool.tile([128, NT * PSH], bf16, tag="hc")
            bias_t = cpool.tile([1, 4 * PSH], bf16, tag="bias")
            ones_t = cpool.tile([1, 128], bf16, tag="ones")
            nc.vector.memset(ones_t[:], 1.0)

            def dma_w8(wt, g, j0, j1):
                nc.sync.dma_start(
                    wt[:, j0:j1, :, :],
                    W8_d.ap()[g][:, j0 * 2 * PSH:j1 * 2 * PSH])

            def dma_a8(q0, q1):
                nc.scalar.dma_start(a8[:, q0:q1, :], A8_d.ap()[:, q0 * B:q1 * B])

            def dma_wbg(q0, q1):
                nc.sync.dma_start(
                    wbg[:, q0 * PSH:q1 * PSH], WB_d.ap()[:, q0 * PSH:q1 * PSH])

            # scalar queue: bias first (needed by the early bias matmuls),
            # then a8 chunks just-in-time for the f-pass.  Everything else
            # rides the sync queue in exact need order, so no stall tricks
            # are required to keep the two queues from preempting each other.
            nc.scalar.dma_start(bias_t[:], B_d.ap()[:])
            for (q0, q1) in ((0, 4), (4, 14), (14, 28), (28, 40)):
                dma_a8(q0, q1)

            # sync queue: all weights in need order.  Descriptor generation
            # costs ~0.6-1.1us of sequencer time per dma_start, so small
            # slabs leave the rings starved between issues: start with one
            # small slab for a fast first matmul, then grow exponentially so
            # ring drain time exceeds issue time.
            for (j0, j1) in ((0, 1), (1, 3), (3, 6), (6, 11), (11, 20)):
                dma_w8(w8f, 0, j0, j1)
            for (j0, j1) in ((0, 3), (3, 7), (7, 13), (13, 20)):
                dma_w8(w8i, 1, j0, j1)
            nc.sync.dma_start(wg8[:, :KG // 2, :, :],
                              WG_d.ap()[:, :(KG // 2) * 2 * PSH])
            nc.sync.dma_start(wg8[:, KG // 2:, :, :],
                              WG_d.ap()[:, (KG // 2) * 2 * PSH:])
            for (q0, q1) in ((0, 5), (5, QB)):
                dma_wbg(q0, q1)
            for (q0, q1) in ((0, QB // 2), (QB // 2, QB)):
                nc.sync.dma_start(
                    ab[:, q0 * B:q1 * B], AB_d.ap()[:, q0 * B:q1 * B])
            nc.sync.dma_start(hc_t[:], H_d.ap()[:])
            for (j0, j1) in ((0, 5), (5, 10), (10, 15), (15, 20)):
                dma_w8(w8o, 2, j0, j1)

            def bias_mm(g, n, start):
                nc.tensor.matmul(
                    psum[g * NT + n][:], ones_t[:],
                    bias_t[:, g * PSH:(g + 1) * PSH],
                    start=start, stop=False)

            # f-pass, DMA-ramp-paced.  Only f's bias matmuls ride here (they
            # fill DMA slack); every other bank opens with its bias matmul
            # (start=True head) immediately before that gate's k-stream, so
            # no cold-clock PE time is wasted on bias work.
            def f_pair(j):
                for n in range(NT):
                    nc.tensor.matmul(
                        psum[0 * NT + n][:],
                        a8[:, 2 * j:2 * j + 2, n * 128:(n + 1) * 128],
                        w8f[:, j, :, :],
                        start=(j == 0), stop=(j == KP - 1), perf_mode=DR)

            f_pair(0)
            bias_mm(0, 0, start=False)
            bias_mm(0, 1, start=False)
            for j in range(1, KP):
                f_pair(j)

            # sigmoid(f) fires at f-stop.
            f_a, i_a, g_a, c_t, th = [], [], [], [], []
            for n in range(NT):
                t = epool.tile([128, PSH], bf16, tag="fa", name=f"fa{n}")
                nc.scalar.activation(t[:], psum[0 * NT + n][:], AF.Sigmoid,
                                     scale=SINV)
                f_a.append(t)

            # i-pass.
            bias_mm(1, 0, start=True)
            bias_mm(1, 1, start=True)
            for j in range(KP):
                for n in range(NT):
                    nc.tensor.matmul(
                        psum[1 * NT + n][:],
                        a8[:, 2 * j:2 * j + 2, n * 128:(n + 1) * 128],
                        w8i[:, j, :, :],
                        start=False, stop=(j == KP - 1), perf_mode=DR)
            for n in range(NT):
                t = epool.tile([128, PSH], bf16, tag="ia", name=f"ia{n}")
                nc.scalar.activation(t[:], psum[1 * NT + n][:], AF.Sigmoid,
                                     scale=SINV)
                i_a.append(t)

            # candidate: fp8 section (x/hu + hidden_s tail, reusing the
            # resident a8), then the bf16 hidden_s head (scaled by the same
            # 2^16 so the bank is scale-uniform).
            bias_mm(3, 0, start=True)
            bias_mm(3, 1, start=True)
            for jj in range(KG):
                for n in range(NT):
                    j = QB // 2 + jj
                    nc.tensor.matmul(
                        psum[3 * NT + n][:],
                        a8[:, 2 * j:2 * j + 2, n * 128:(n + 1) * 128],
                        wg8[:, jj, :, :],
                        start=False, stop=False, perf_mode=DR)
            for q in range(QB):
                for n in range(NT):
                    nc.tensor.matmul(
                        psum[3 * NT + n][:],
                        ab[:, q * B + n * 128:q * B + (n + 1) * 128],
                        wbg[:, q * PSH:(q + 1) * PSH],
                        start=False, stop=(q == QB - 1))
            for n in range(NT):
                t = epool.tile([128, PSH], bf16, tag="ga", name=f"ga{n}")
                nc.scalar.activation(t[:], psum[3 * NT + n][:], AF.Tanh,
                                     scale=SINV)
                g_a.append(t)
            for n in range(NT):
                fhc = epool.tile([128, PSH], f32, tag="fhc", name=f"fhc{n}")
                nc.vector.tensor_mul(
                    fhc[:], f_a[n][:], hc_t[:, n * PSH:(n + 1) * PSH])
                ig = epool.tile([128, PSH], f32, tag="ig", name=f"ig{n}")
                nc.vector.tensor_mul(ig[:], i_a[n][:], g_a[n][:])
                ct = epool.tile([128, PSH], bf16, tag="ct", name=f"ct{n}")
                nc.vector.tensor_add(ct[:], fhc[:], ig[:])
                c_t.append(ct)
                nc.sync.dma_start(CO_d.ap()[n], ct[:])
            for n in range(NT):
                t = epool.tile([128, PSH], bf16, tag="th", name=f"th{n}")
                nc.scalar.activation(t[:], c_t[n][:], AF.Tanh)
                th.append(t)

            # o-pass, n-major: batch-tile 0's bank closes first so its
            # sigmoid/mul/store hide under batch-tile 1's matmuls.  Batch-tile
            # 1 is column-split; the high half accumulates in the candidate
            # n=0 bank (free after tanh(g0)) so the low half's epilogue
            # doesn't serialize against the high half through a shared PSUM
            # tile.  Each bank's k-order is rotated so its stop matmul uses
            # pair 0 (whose weight slab has long been resident): the w8o
            # stream arrives just-in-time, the scheduler interleaves the
            # banks slab-by-slab, and without the rotation all three stops
            # would land together at the last slab's arrival, collapsing all
            # three epilogues onto the kernel tail.
            o_a = [
                epool.tile([128, PSH], bf16, tag="oa", name=f"oa{n}")
                for n in range(NT)
            ]
            os_t = [
                epool.tile([128, PSH], bf16, tag="ost", name=f"ost{n}")
                for n in range(NT)
            ]
            HP = PSH // 2
            ps_o1h = psum[3 * NT + 0]
            korder = list(range(1, KP)) + [0]
            bias_mm(2, 0, start=True)
            bias_mm(2, 1, start=True)
            nc.tensor.matmul(
                ps_o1h[:, :HP], ones_t[:],
                bias_t[:, 2 * PSH + HP:3 * PSH],
                start=True, stop=False)
            for j in korder:
                nc.tensor.matmul(
                    psum[2 * NT + 0][:],
                    a8[:, 2 * j:2 * j + 2, 0:128],
                    w8o[:, j, :, :],
                    start=False, stop=(j == 0), perf_mode=DR)
            for j in korder:
                nc.tensor.matmul(
                    psum[2 * NT + 1][:, :HP],
                    a8[:, 2 * j:2 * j + 2, 128:256],
                    w8o[:, j, :, :HP],
                    start=False, stop=(j == 0), perf_mode=DR)
            # n=0's full epilogue hides under n=1's low-half matmuls.
            nc.scalar.activation(o_a[0][:], psum[2 * NT + 0][:],
                                 AF.Sigmoid, scale=SINV)
            nc.vector.tensor_mul(os_t[0][:], o_a[0][:], th[0][:])
            nc.sync.dma_start(OS_d.ap()[0], os_t[0][:])
            for j in korder:
                nc.tensor.matmul(
                    ps_o1h[:, :HP],
                    a8[:, 2 * j:2 * j + 2, 128:256],
                    w8o[:, j, :, HP:],
                    start=False, stop=(j == 0), perf_mode=DR)
            # n=1 low half drains while the high half computes.
            nc.scalar.activation(o_a[1][:, :HP], psum[2 * NT + 1][:, :HP],
                                 AF.Sigmoid, scale=SINV)
            nc.vector.tensor_mul(
                os_t[1][:, :HP], o_a[1][:, :HP], th[1][:, :HP])
            nc.sync.dma_start(OS_d.ap()[1][:, :HP], os_t[1][:, :HP])
            # final high half: store split across both HWDGE queues.
            nc.scalar.activation(o_a[1][:, HP:], ps_o1h[:, :HP],
                                 AF.Sigmoid, scale=SINV)
            nc.vector.tensor_mul(os_t[1][:, HP:], o_a[1][:, HP:], th[1][:, HP:])
            HQ = HP // 2
            nc.sync.dma_start(OS_d.ap()[1][:, HP:HP + HQ],
                              os_t[1][:, HP:HP + HQ])
            nc.scalar.dma_start(OS_d.ap()[1][:, HP + HQ:],
                                os_t[1][:, HP + HQ:])

    nc.compile()
    return nc


def _get_nc():
    if "nc" not in _cache:
        _cache["nc"] = _build()
    return _cache["nc"]


def _prep_in_maps(x, hidden_s, hidden_u, hidden_c, Wd, Wu, Ww, Bd, Bu, Bw):
    # Activations, transposed: A_T[k, n], k = [hs (4096) | hu (512) | x (512)]
    A = np.concatenate(
        [hidden_s.reshape(B, S2), hidden_u, x], axis=1)                # [B, QC]
    A_kt = (A.T * SA).reshape(QT, 128, B)                              # [q,p,n]
    A8 = np.ascontiguousarray(
        A_kt.astype(F8).transpose(1, 0, 2)).reshape(128, QT * B)
    AB = np.ascontiguousarray(
        A_kt[:QB].astype(BF).transpose(1, 0, 2)).reshape(128, QB * B)

    # Weights, transposed to [k, p]; contraction order [Wu | Ww | -Wd].
    # Everything is pre-scaled by SW (and activations by SA) so every PSUM
    # bank carries the same 2^16 scale; bf16 scaling by powers of 2 is exact.
    WuT = Wu.reshape(4, S2, S2).transpose(0, 2, 1)                     # [4,S2,S2]
    WwT = Ww.reshape(4, S2, U).transpose(0, 2, 1)                      # [4,U,S2]
    WdT = (-Wd.reshape(4, S2, I)).transpose(0, 2, 1)                   # [4,I,S2]
    WT = np.concatenate([WuT, WwT, WdT], axis=1) * SW                  # [4,QC,S2]
    W8_all = WT[:3].astype(F8)                                         # [3,QC,S2]
    WG_all = WT[3][QB * 128:].astype(F8)                               # [1024,S2]
    WB_all = WT[3][:QB * 128].astype(BF)                               # [4096,S2]

    bias = (Bw + Bd + Bu).reshape(4, S2).astype(np.float64) * (SA * SW)
    hc = hidden_c.reshape(NT, 128, S2)

    in_maps = []
    for c in range(NCORES):
        p0 = c * PSH
        W8_c = np.ascontiguousarray(
            W8_all[:, :, p0:p0 + PSH].reshape(3, KP, 2, 128, PSH)
            .transpose(0, 3, 1, 2, 4)).reshape(3, 128, KP * 2 * PSH)
        WG_c = np.ascontiguousarray(
            WG_all[:, p0:p0 + PSH].reshape(KG, 2, 128, PSH)
            .transpose(2, 0, 1, 3)).reshape(128, KG * 2 * PSH)
        WB_c = np.ascontiguousarray(
            WB_all[:, p0:p0 + PSH].reshape(QB, 128, PSH)
            .transpose(1, 0, 2)).reshape(128, QB * PSH)
        b_c = np.ascontiguousarray(
            bias[:, p0:p0 + PSH]).reshape(1, 4 * PSH).astype(BF)
        h_c = np.ascontiguousarray(
            hc[..., p0:p0 + PSH].transpose(1, 0, 2)).reshape(
                128, NT * PSH).astype(BF)
        in_maps.append({"A8": A8, "AB": AB, "W8": W8_c, "WG8": WG_c,
                        "WB": WB_c, "BIAS": b_c, "HC": h_c})
    return in_maps


def _run(inputs, trace=False, trace_kwargs=None):
    from concourse.bass_utils import run_bass_kernel_spmd

    nc = _get_nc()
    in_maps = _prep_in_maps(**inputs)
    res = run_bass_kernel_spmd(
        nc, in_maps, core_ids=list(range(NCORES)),
        trace=trace, **(trace_kwargs or {}))

    out_s = np.empty((B, S2), np.float32)
    c_out = np.empty((B, S2), np.float32)
    for c in range(NCORES):
        p0 = c * PSH
        out_s[:, p0:p0 + PSH] = res.results[c]["OS"].astype(
            np.float32).reshape(B, PSH)
        c_out[:, p0:p0 + PSH] = res.results[c]["CO"].astype(
            np.float32).reshape(B, PSH)
    return (out_s.reshape(B, 64, 64), c_out.reshape(B, 64, 64)), res


def kernel(**inputs):
    inputs = {k: np.asarray(v) for k, v in inputs.items()}
    (out_s, c_out), _ = _run(inputs)
    return (out_s, c_out)


# revision 16
# speedup vs baseline: 1.0087x; 1.0087x over previous
"""Trainium2 Bass kernel for nn_LstmCellS (matrix-state LSTM cell).

Math (per gate g in [f, i, o, c]):
    pre[g] = hidden_u @ Ww[g]^T - x @ Wd[g]^T + hidden_s @ Wu[g]^T + (Bw+Bd+Bu)[g]
    f, i, o = sigmoid(pre[0..2]);  gg = tanh(pre[3])
    c     = f * hidden_c + i * gg
    out_s = o * tanh(c)

Sharding: tensor-parallel over the output axis p (flattened (a, b), S^2 = 4096
-> 512 per core).  Every core sees the full batch and full contraction but only
its 512-wide output slice of every gate, so the whole LSTM epilogue is local —
no collectives.  Host concatenates the 8 output slices.

Mixed precision (rel err ~1.77e-2 vs the 2e-2 budget; inputs deterministic):
  * f/i/o gates run fp8-e4m3 DoubleRow (2 contraction rows per PE cell/cycle).
  * The candidate (tanh) gate keeps bf16 for the first QB=20 hidden_s k-tiles
    and fp8 DoubleRow for the rest, reusing the resident fp8 activations.
  * fp8 operands pre-scaled on host (acts x16, weights x4096); bf16 operands
    carry the same power-of-two scales so every PSUM bank is uniformly
    2^16-scaled and the epilogue activation applies scale=2^-16.

Schedule (v2, from trace analysis of the v1 kernel):
  * ~7us of framework preamble is fixed; first DMA packets land ~8.4us.  DMA
    sustains ~400 GB/s; the PE matmul stream (~40us warm) is the critical
    path, so the schedule exists to keep the PE fed from the first packet on.
  * DMA issue order == need order.  sync queue carries all weights
    (w8f slabs fine-grained for JIT start, then w8i, wg8, wbg, w8o, then
    result stores); scalar queue carries bias + a8 chunks (JIT for the
    f-pass), then stalls behind the f-sigmoids so ab/hc cannot preempt w8i.
  * The f-pass is DMA-ramp-paced, so the 8 bias matmuls (one K=1 matmul per
    PSUM bank) are slotted between the first f-pairs where the PE has slack;
    banks not yet opened get their bias matmul as the start=True group head.
  * Only 5 dummy matmuls bridge engine-free (~7us) to first-weights (~9us);
    cold-clock matmuls are faster than the early DMA pace anyway.
  * Gate order f, i, candidate, o: c = f*hc + i*g runs on scalar/vector under
    the o matmuls.  o is n-major; batch-tile 1 is column-split so its first
    half's sigmoid/mul/store hides under the second half's matmuls.
"""

import sys

for _p in ("/root/.axon_site/_ro/trn_rl_repo", "/opt/trn_rl_repo"):
    if _p not in sys.path:
        sys.path.append(_p)

import ml_dtypes
import numpy as np

B = 256          # batch
S2 = 4096        # S*S (flattened matrix state)
U = 512          # hidden_u size
I = 512          # input size
QC = S2 + U + I  # contraction length (5120)
QT = QC // 128   # contraction tiles (40)
KP = QT // 2     # fp8 DoubleRow k-pairs (20)
QB = 10          # candidate-gate bf16 k-tiles (high-variance section)
KG = (QT - QB) // 2  # candidate-gate fp8 k-pairs (10)
NT = B // 128    # batch tiles (2)
NCORES = 8
PSH = S2 // NCORES  # output slice per core (512)

SA = 16.0        # fp8 activation scale
SW = 4096.0      # fp8 weight scale
SINV = 1.0 / (SA * SW)

F8 = ml_dtypes.float8_e4m3  # TRN float8e4: bias 7, max normal +-240
BF = ml_dtypes.bfloat16

_cache: dict = {}


def _build():
    """Build and compile the per-core Bass module (same NEFF on all cores)."""
    import concourse.tile as tile
    import concourse.mybir as mybir
    from concourse import bacc

    f32 = mybir.dt.float32
    bf16 = mybir.dt.bfloat16
    fp8 = mybir.dt.float8e4
    AF = mybir.ActivationFunctionType
    DR = mybir.MatmulPerfMode.DoubleRow

    nc = bacc.Bacc("TRN2", target_bir_lowering=False, debug=False,
                   enable_asserts=False, num_devices=NCORES)

    A8_d = nc.dram_tensor("A8", [128, QT * B], fp8, kind="ExternalInput")
    AB_d = nc.dram_tensor("AB", [128, QB * B], bf16, kind="ExternalInput")
    W8_d = nc.dram_tensor("W8", [3, 128, KP * 2 * PSH], fp8, kind="ExternalInput")
    WG_d = nc.dram_tensor("WG8", [128, KG * 2 * PSH], fp8, kind="ExternalInput")
    WB_d = nc.dram_tensor("WB", [128, QB * PSH], bf16, kind="ExternalInput")
    B_d = nc.dram_tensor("BIAS", [1, 4 * PSH], bf16, kind="ExternalInput")
    H_d = nc.dram_tensor("HC", [128, NT * PSH], bf16, kind="ExternalInput")
    OS_d = nc.dram_tensor("OS", [NT, 128, PSH], bf16, kind="ExternalOutput")
    CO_d = nc.dram_tensor("CO", [NT, 128, PSH], bf16, kind="ExternalOutput")

    with tile.TileContext(nc) as tc:
        with (
            tc.tile_pool(name="apool", bufs=1) as apool,
            tc.tile_pool(name="wpool", bufs=1) as wpool,
            tc.tile_pool(name="cpool", bufs=1) as cpool,
            tc.tile_pool(name="epool", bufs=2) as epool,
            tc.tile_pool(name="pspool", bufs=1, space="PSUM") as pspool,
        ):
            # PSUM accumulators: bank per (gate, batch-tile); gate ids f0 i1 o2 g3
            psum = [
                pspool.tile([128, PSH], f32, tag=f"ps{g}_{n}", name=f"ps{g}_{n}")
                for g in range(4) for n in range(NT)
            ]

            # Contiguous dummy-matmul block: ~3.8us of back-to-back PE work
            # flips the HAM clock gate to 8/8 by ~11us, so the whole f-pass
            # runs at 2.4 GHz instead of 1.2 (a sparse early stream with DMA
            # waits never sustains a full HAM busy window — measured warm-up
            # then happens only at ~21us, leaving the f-pass cold-PE-bound).
            # They pollute psum[0], which the f start=True head clears.
            scr = cpool.tile([128, 128 + PSH], bf16, tag="scr")
            nc.vector.memset(scr[:], 0.0)
            for _ in range(9):
                nc.tensor.matmul(
                    psum[0][:], scr[:, :128], scr[:, 128:],
                    start=True, stop=True, skip_group_check=True)

            # SBUF-resident operands.
            a8 = apool.tile([128, QT, B], fp8, tag="a8", name="a8")
            ab = apool.tile([128, QB * B], bf16, tag="ab", name="ab")
            wg8 = wpool.tile([128, KG, 2, PSH], fp8, tag="wg8", name="wg8")
            w8f = wpool.tile([128, KP, 2, PSH], fp8, tag="w8f", name="w8f")
            w8i = wpool.tile([128, KP, 2, PSH], fp8, tag="w8i", name="w8i")
            w8o = wpool.tile([128, KP, 2, PSH], fp8, tag="w8o", name="w8o")
            wbg = wpool.tile([128, QB * PSH], bf16, tag="wbg", name="wbg")
            hc_t = cpool.tile([128, NT * PSH], bf16, tag="hc")
            bias_t = cpool.tile([1, 4 * PSH], bf16, tag="bias")
            ones_t = cpool.tile([1, 128], bf16, tag="ones")
            nc.vector.memset(ones_t[:], 1.0)

            def dma_w8(wt, g, j0, j1):
                nc.sync.dma_start(
                    wt[:, j0:j1, :, :],
                    W8_d.ap()[g][:, j0 * 2 * PSH:j1 * 2 * PSH])

            def dma_a8(q0, q1):
                nc.scalar.dma_start(a8[:, q0:q1, :], A8_d.ap()[:, q0 * B:q1 * B])

            def dma_wbg(q0, q1):
                nc.sync.dma_start(
                    wbg[:, q0 * PSH:q1 * PSH], WB_d.ap()[:, q0 * PSH:q1 * PSH])

            # scalar queue: bias first (needed by the early bias matmuls),
            # then a8 chunks just-in-time for the f-pass.  Everything else
            # rides the sync queue in exact need order, so no stall tricks
            # are required to keep the two queues from preempting each other.
            nc.scalar.dma_start(bias_t[:], B_d.ap()[:])
            for (q0, q1) in ((0, 4), (4, 14), (14, 28), (28, 40)):
                dma_a8(q0, q1)

            # sync queue: all weights in need order.  Descriptor generation
            # costs ~0.6-1.1us of sequencer time per dma_start, so small
            # slabs leave the rings starved between issues: start with one
            # small slab for a fast first matmul, then grow exponentially so
            # ring drain time exceeds issue time.
            for (j0, j1) in ((0, 1), (1, 3), (3, 6), (6, 10), (10, 13),
                             (13, 16), (16, 18), (18, 20)):
                dma_w8(w8f, 0, j0, j1)
            for (j0, j1) in ((0, 3), (3, 7), (7, 13), (13, 20)):
                dma_w8(w8i, 1, j0, j1)
            nc.sync.dma_start(wg8[:, :KG // 2, :, :],
                              WG_d.ap()[:, :(KG // 2) * 2 * PSH])
            nc.sync.dma_start(wg8[:, KG // 2:, :, :],
                              WG_d.ap()[:, (KG // 2) * 2 * PSH:])
            for (q0, q1) in ((0, 5), (5, QB)):
                dma_wbg(q0, q1)
            for (q0, q1) in ((0, QB // 2), (QB // 2, QB)):
                nc.sync.dma_start(
                    ab[:, q0 * B:q1 * B], AB_d.ap()[:, q0 * B:q1 * B])
            nc.sync.dma_start(hc_t[:], H_d.ap()[:])
            for (j0, j1) in ((0, 5), (5, 10), (10, 15), (15, 20)):
                dma_w8(w8o, 2, j0, j1)

            def bias_mm(g, n, start):
                nc.tensor.matmul(
                    psum[g * NT + n][:], ones_t[:],
                    bias_t[:, g * PSH:(g + 1) * PSH],
                    start=start, stop=False)

            # f-pass, DMA-ramp-paced.  Only f's bias matmuls ride here (they
            # fill DMA slack); every other bank opens with its bias matmul
            # (start=True head) immediately before that gate's k-stream, so
            # no cold-clock PE time is wasted on bias work.
            def f_pair(j):
                for n in range(NT):
                    nc.tensor.matmul(
                        psum[0 * NT + n][:],
                        a8[:, 2 * j:2 * j + 2, n * 128:(n + 1) * 128],
                        w8f[:, j, :, :],
                        start=(j == 0), stop=(j == KP - 1), perf_mode=DR)

            f_pair(0)
            bias_mm(0, 0, start=False)
            bias_mm(0, 1, start=False)
            for j in range(1, KP):
                f_pair(j)

            # sigmoid(f) fires at f-stop.
            f_a, i_a, g_a, c_t, th = [], [], [], [], []
            for n in range(NT):
                t = epool.tile([128, PSH], bf16, tag="fa", name=f"fa{n}")
                nc.scalar.activation(t[:], psum[0 * NT + n][:], AF.Sigmoid,
                                     scale=SINV)
                f_a.append(t)

            # i-pass.
            bias_mm(1, 0, start=True)
            bias_mm(1, 1, start=True)
            for j in range(KP):
                for n in range(NT):
                    nc.tensor.matmul(
                        psum[1 * NT + n][:],
                        a8[:, 2 * j:2 * j + 2, n * 128:(n + 1) * 128],
                        w8i[:, j, :, :],
                        start=False, stop=(j == KP - 1), perf_mode=DR)
            for n in range(NT):
                t = epool.tile([128, PSH], bf16, tag="ia", name=f"ia{n}")
                nc.scalar.activation(t[:], psum[1 * NT + n][:], AF.Sigmoid,
                                     scale=SINV)
                i_a.append(t)

            # candidate: fp8 section (x/hu + hidden_s tail, reusing the
            # resident a8), then the bf16 hidden_s head (scaled by the same
            # 2^16 so the bank is scale-uniform).
            bias_mm(3, 0, start=True)
            bias_mm(3, 1, start=True)
            for jj in range(KG):
                for n in range(NT):
                    j = QB // 2 + jj
                    nc.tensor.matmul(
                        psum[3 * NT + n][:],
                        a8[:, 2 * j:2 * j + 2, n * 128:(n + 1) * 128],
                        wg8[:, jj, :, :],
                        start=False, stop=False, perf_mode=DR)
            for q in range(QB):
                for n in range(NT):
                    nc.tensor.matmul(
                        psum[3 * NT + n][:],
                        ab[:, q * B + n * 128:q * B + (n + 1) * 128],
                        wbg[:, q * PSH:(q + 1) * PSH],
                        start=False, stop=(q == QB - 1))
            for n in range(NT):
                t = epool.tile([128, PSH], bf16, tag="ga", name=f"ga{n}")
                nc.scalar.activation(t[:], psum[3 * NT + n][:], AF.Tanh,
                                     scale=SINV)
                g_a.append(t)
            for n in range(NT):
                fhc = epool.tile([128, PSH], f32, tag="fhc", name=f"fhc{n}")
                nc.vector.tensor_mul(
                    fhc[:], f_a[n][:], hc_t[:, n * PSH:(n + 1) * PSH])
                ig = epool.tile([128, PSH], f32, tag="ig", name=f"ig{n}")
                nc.vector.tensor_mul(ig[:], i_a[n][:], g_a[n][:])
                ct = epool.tile([128, PSH], bf16, tag="ct", name=f"ct{n}")
                nc.vector.tensor_add(ct[:], fhc[:], ig[:])
                c_t.append(ct)
                nc.sync.dma_start(CO_d.ap()[n], ct[:])
            for n in range(NT):
                t = epool.tile([128, PSH], bf16, tag="th", name=f"th{n}")
                nc.scalar.activation(t[:], c_t[n][:], AF.Tanh)
                th.append(t)

            # o-pass, n-major: batch-tile 0's bank closes first so its
            # sigmoid/mul/store hide under batch-tile 1's matmuls.  Batch-tile
            # 1 is column-split; the high half accumulates in the candidate
            # n=0 bank (free after tanh(g0)) so the low half's epilogue
            # doesn't serialize against the high half through a shared PSUM
            # tile.  Each bank's k-order is rotated so its stop matmul uses
            # pair 0 (whose weight slab has long been resident): the w8o
            # stream arrives just-in-time, the scheduler interleaves the
            # banks slab-by-slab, and without the rotation all three stops
            # would land together at the last slab's arrival, collapsing all
            # three epilogues onto the kernel tail.
            o_a = [
                epool.tile([128, PSH], bf16, tag="oa", name=f"oa{n}")
                for n in range(NT)
            ]
            os_t = [
                epool.tile([128, PSH], bf16, tag="ost", name=f"ost{n}")
                for n in range(NT)
            ]
            HP = PSH // 2
            ps_o1h = psum[3 * NT + 0]
            korder = list(range(1, KP)) + [0]
            bias_mm(2, 0, start=True)
            bias_mm(2, 1, start=True)
            nc.tensor.matmul(
                ps_o1h[:, :HP], ones_t[:],
                bias_t[:, 2 * PSH + HP:3 * PSH],
                start=True, stop=False)
            for j in korder:
                nc.tensor.matmul(
                    psum[2 * NT + 0][:],
                    a8[:, 2 * j:2 * j + 2, 0:128],
                    w8o[:, j, :, :],
                    start=False, stop=(j == 0), perf_mode=DR)
            for j in korder:
                nc.tensor.matmul(
                    psum[2 * NT + 1][:, :HP],
                    a8[:, 2 * j:2 * j + 2, 128:256],
                    w8o[:, j, :, :HP],
                    start=False, stop=(j == 0), perf_mode=DR)
            # n=0's full epilogue hides under n=1's low-half matmuls.
            nc.scalar.activation(o_a[0][:], psum[2 * NT + 0][:],
                                 AF.Sigmoid, scale=SINV)
            nc.vector.tensor_mul(os_t[0][:], o_a[0][:], th[0][:])
            nc.sync.dma_start(OS_d.ap()[0], os_t[0][:])
            for j in korder:
                nc.tensor.matmul(
                    ps_o1h[:, :HP],
                    a8[:, 2 * j:2 * j + 2, 128:256],
                    w8o[:, j, :, HP:],
                    start=False, stop=(j == 0), perf_mode=DR)
            # n=1 low half drains while the high half computes.
            nc.scalar.activation(o_a[1][:, :HP], psum[2 * NT + 1][:, :HP],
                                 AF.Sigmoid, scale=SINV)
            nc.vector.tensor_mul(
                os_t[1][:, :HP], o_a[1][:, :HP], th[1][:, :HP])
            nc.sync.dma_start(OS_d.ap()[1][:, :HP], os_t[1][:, :HP])
            # final high half: store split across both HWDGE queues.
            nc.scalar.activation(o_a[1][:, HP:], ps_o1h[:, :HP],
                                 AF.Sigmoid, scale=SINV)
            nc.vector.tensor_mul(os_t[1][:, HP:], o_a[1][:, HP:], th[1][:, HP:])
            HQ = HP // 2
            nc.sync.dma_start(OS_d.ap()[1][:, HP:HP + HQ],
                              os_t[1][:, HP:HP + HQ])
            nc.scalar.dma_start(OS_d.ap()[1][:, HP + HQ:],
                                os_t[1][:, HP + HQ:])

    nc.compile()
    return nc


def _get_nc():
    if "nc" not in _cache:
        _cache["nc"] = _build()
    return _cache["nc"]


def _prep_in_maps(x, hidden_s, hidden_u, hidden_c, Wd, Wu, Ww, Bd, Bu, Bw):
    # Activations, transposed: A_T[k, n], k = [hs (4096) | hu (512) | x (512)]
    A = np.concatenate(
        [hidden_s.reshape(B, S2), hidden_u, x], axis=1)                # [B, QC]
    A_kt = (A.T * SA).reshape(QT, 128, B)                              # [q,p,n]
    A8 = np.ascontiguousarray(
        A_kt.astype(F8).transpose(1, 0, 2)).reshape(128, QT * B)
    AB = np.ascontiguousarray(
        A_kt[:QB].astype(BF).transpose(1, 0, 2)).reshape(128, QB * B)

    # Weights, transposed to [k, p]; contraction order [Wu | Ww | -Wd].
    # Everything is pre-scaled by SW (and activations by SA) so every PSUM
    # bank carries the same 2^16 scale; bf16 scaling by powers of 2 is exact.
    WuT = Wu.reshape(4, S2, S2).transpose(0, 2, 1)                     # [4,S2,S2]
    WwT = Ww.reshape(4, S2, U).transpose(0, 2, 1)                      # [4,U,S2]
    WdT = (-Wd.reshape(4, S2, I)).transpose(0, 2, 1)                   # [4,I,S2]
    WT = np.concatenate([WuT, WwT, WdT], axis=1) * SW                  # [4,QC,S2]
    W8_all = WT[:3].astype(F8)                                         # [3,QC,S2]
    WG_all = WT[3][QB * 128:].astype(F8)                               # [1024,S2]
    WB_all = WT[3][:QB * 128].astype(BF)                               # [4096,S2]

    bias = (Bw + Bd + Bu).reshape(4, S2).astype(np.float64) * (SA * SW)
    hc = hidden_c.reshape(NT, 128, S2)

    in_maps = []
    for c in range(NCORES):
        p0 = c * PSH
        W8_c = np.ascontiguousarray(
            W8_all[:, :, p0:p0 + PSH].reshape(3, KP, 2, 128, PSH)
            .transpose(0, 3, 1, 2, 4)).reshape(3, 128, KP * 2 * PSH)
        WG_c = np.ascontiguousarray(
            WG_all[:, p0:p0 + PSH].reshape(KG, 2, 128, PSH)
            .transpose(2, 0, 1, 3)).reshape(128, KG * 2 * PSH)
        WB_c = np.ascontiguousarray(
            WB_all[:, p0:p0 + PSH].reshape(QB, 128, PSH)
            .transpose(1, 0, 2)).reshape(128, QB * PSH)
        b_c = np.ascontiguousarray(
            bias[:, p0:p0 + PSH]).reshape(1, 4 * PSH).astype(BF)
        h_c = np.ascontiguousarray(
            hc[..., p0:p0 + PSH].transpose(1, 0, 2)).reshape(
                128, NT * PSH).astype(BF)
        in_maps.append({"A8": A8, "AB": AB, "W8": W8_c, "WG8": WG_c,
                        "WB": WB_c, "BIAS": b_c, "HC": h_c})
    return in_maps


def _run(inputs, trace=False, trace_kwargs=None):
    from concourse.bass_utils import run_bass_kernel_spmd

    nc = _get_nc()
    in_maps = _prep_in_maps(**inputs)
    res = run_bass_kernel_spmd(
        nc, in_maps, core_ids=list(range(NCORES)),
        trace=trace, **(trace_kwargs or {}))

    out_s = np.empty((B, S2), np.float32)
    c_out = np.empty((B, S2), np.float32)
    for c in range(NCORES):
        p0 = c * PSH
        out_s[:, p0:p0 + PSH] = res.results[c]["OS"].astype(
            np.float32).reshape(B, PSH)
        c_out[:, p0:p0 + PSH] = res.results[c]["CO"].astype(
            np.float32).reshape(B, PSH)
    return (out_s.reshape(B, 64, 64), c_out.reshape(B, 64, 64)), res


def kernel(**inputs):
    inputs = {k: np.asarray(v) for k, v in inputs.items()}
    (out_s, c_out), _ = _run(inputs)
    return (out_s, c_out)
